# revision 3
# baseline (speedup 1.0000x reference)
"""GGNN message passing + bilinear readout on 8 TRN2 NeuronCores.

Problem: nn_BaselineModel_36687610642509 (gnn_message_passing).

reference:
    for 8 iters:  per_edge = einsum('sd,edh->seh', h, W_msg)
                  messages = einsum('ste,seh->th', edge, per_edge) + b_msg
                  h = GRU(h, messages)          (Wi, Wh, b_gru)
    logits = einsum('id,de,je->ij', h, A_readout, h)

Distribution (destination-sharded, per the sharding hint): core k owns
destination nodes t_k = [256k, 256k+256).
  - edge shard edge[:, t_k, :] resident in SBUF, tiles [(e, sb)] of
    [128 senders, 256 own dests].
  - h replicated in normal [s, d] layout (16 blocks of [128, 128]).
  - step1: M_e[d, t] = sum_s h[s,d] edge_e[s,t]   (h blocks stationary,
    edge moving). The W-transform commutes with the sender aggregation,
    so per_edge never needs to be materialized:
  - step2: msgs[h', t] = sum_e W_e[d,h']^T M_e[d,t]  (static W stationary)
  - GRU updates own shard h^T [D, S]; transpose (PE) to normal layout and
    AllGather (64 KiB payload, ~5 us) for the next iteration — vs the
    sender-sharded design's 512 KiB ReduceScatter (~13 us) of partial
    messages.
  - final iteration all-gathers h^T instead, feeding the bilinear readout.

PSUM note: has_written clears are bank-granular, so two matmul
accumulation groups must never interleave within a PSUM bank. M_e region
e lives at cols e*S of a [128, 2048] PSUM tile: regions 2b/2b+1 share
bank b -> step1 runs even e first, then odd e (one open group per bank;
cross-bank interleaving is safe — verified on HW).

fp8 mode (default): edge is pre-scaled by 2048 into e4m3 (values ~U[0,1])
and 1/2048 is folded into W_msg; h is cast to e4m3 for step1 only (GRU
state stays bf16). step1 then runs DoubleRow matmuls (2 sender blocks
per instruction, 2 MACs/cell/cycle) — halves step1 time. Measured rel
err ~1.5e-2 (numpy-sim 1.54e-2), inside the 2e-2 gate; bf16 fallback
(FP8=False) sits at 8.0e-3.
"""

import sys

for _p in ("/opt/trn_rl_repo",):
    if _p not in sys.path:
        sys.path.insert(0, _p)

import numpy as np
import ml_dtypes

import concourse.bacc as bacc
import concourse.tile as tile
import concourse.mybir as mybir
from concourse import bass_utils

dt = mybir.dt
AF = mybir.ActivationFunctionType
PM = mybir.MatmulPerfMode

N_CORES = 8
N = 2048
D = 128
E = 8
ITERS = 8
S = N // N_CORES          # 256 own destinations per core
NSB = N // D              # 16 sender blocks
RG = [list(range(N_CORES))]

FP8 = False  # flipped to True once validated on HW
EDGE_SCALE = 2048.0


def build_nc(reps=1, fp8=FP8):
    nc = bacc.Bacc("TRN2", target_bir_lowering=False, debug=False,
                   num_devices=N_CORES)

    edt = dt.float8e4 if fp8 else dt.bfloat16
    # edge shard, rows e*N + s, cols = own dests
    edgek = nc.dram_tensor("edgek", [E * N, S], edt, kind="ExternalInput")
    h0n = nc.dram_tensor("h0n", [N, D], edt, kind="ExternalInput")
    h0t = nc.dram_tensor("h0t", [D, S], dt.bfloat16, kind="ExternalInput")
    wmsg = nc.dram_tensor("wmsg", [D, E * D], dt.bfloat16, kind="ExternalInput")
    wi = nc.dram_tensor("wi", [D, 3 * D], dt.bfloat16, kind="ExternalInput")
    wh = nc.dram_tensor("wh", [D, 3 * D], dt.bfloat16, kind="ExternalInput")
    bias = nc.dram_tensor("bias", [D, 3], dt.float32, kind="ExternalInput")
    aro = nc.dram_tensor("aro", [D, D], dt.bfloat16, kind="ExternalInput")
    ident = nc.dram_tensor("ident", [D, D], dt.bfloat16, kind="ExternalInput")
    out = nc.dram_tensor("out", [S, N], dt.float32, kind="ExternalOutput")

    with tile.TileContext(nc) as tc:
        with (
            tc.tile_pool(name="const", bufs=1) as cpool,
            tc.tile_pool(name="hb", bufs=2) as hpool,
            tc.tile_pool(name="sb", bufs=2) as spool,
            tc.tile_pool(name="stage", bufs=4) as stpool,
            tc.tile_pool(name="mps", bufs=1, space="PSUM") as mps_pool,
            tc.tile_pool(name="sps", bufs=3, space="PSUM") as sps_pool,
            tc.tile_pool(name="dram", bufs=2, space="DRAM") as dram,
        ):
            for rep in range(reps):
                # ---- small constants first (so edge DMAs don't block them)
                wmsg_sb = cpool.tile([D, E * D], dt.bfloat16, tag="wmsg")
                nc.sync.dma_start(wmsg_sb[:], wmsg.ap())
                wi_sb = cpool.tile([D, 3 * D], dt.bfloat16, tag="wi")
                nc.sync.dma_start(wi_sb[:], wi.ap())
                wh_sb = cpool.tile([D, 3 * D], dt.bfloat16, tag="wh")
                nc.sync.dma_start(wh_sb[:], wh.ap())
                bias_sb = cpool.tile([D, 3], dt.float32, tag="bias")
                nc.sync.dma_start(bias_sb[:], bias.ap())
                aro_sb = cpool.tile([D, D], dt.bfloat16, tag="aro")
                nc.sync.dma_start(aro_sb[:], aro.ap())
                id_sb = cpool.tile([D, D], dt.bfloat16, tag="ident")
                nc.sync.dma_start(id_sb[:], ident.ap())
                hT = spool.tile([D, S], dt.bfloat16, tag="hT")
                nc.sync.dma_start(hT[:], h0t.ap())

                # replicated h, normal layout. fp8: 8 pair-tiles
                # [128, 2, 128] (DoubleRow stationary); bf16: 16 [128, 128]
                if fp8:
                    hblk = []
                    for p in range(NSB // 2):
                        t = hpool.tile([D, 2, D], edt, tag=f"hpair{p}",
                                       name=f"hpair{p}")
                        nc.sync.dma_start(
                            t[:],
                            h0n.ap()[2 * p * D:(2 * p + 2) * D, :]
                            .rearrange("(a p) c -> p a c", p=D))
                        hblk.append(t)
                else:
                    hblk = []
                    for sb in range(NSB):
                        t = hpool.tile([D, D], edt, tag=f"hblk{sb}",
                                       name=f"hblk{sb}")
                        nc.sync.dma_start(t[:],
                                          h0n.ap()[sb * D:(sb + 1) * D, :])
                        hblk.append(t)

                # ---- edge shard: 32 chase-ordered DMAs of [128, 4, 256]
                # (4 sender-blocks each); issue order matches first use.
                edge_sb = {}
                for e in range(E):
                    for g in range(4):
                        edge_sb[(e, g)] = cpool.tile(
                            [D, 4, S], edt, tag=f"edge{e}_{g}",
                            name=f"edge{e}_{g}")
                e_order = [0, 2, 4, 6, 1, 3, 5, 7]
                for g in range(4):
                    for e in e_order:
                        src = edgek.ap()[e * N + g * 512:e * N + (g + 1) * 512, :]
                        nc.sync.dma_start(
                            edge_sb[(e, g)][:],
                            src.rearrange("(a p) c -> p a c", p=D))

                def edge_ap(e, sb):
                    g, a = sb // 4, sb % 4
                    return edge_sb[(e, g)][:, a, :]

                def edge_pair_ap(e, p):
                    g, a = (2 * p) // 4, (2 * p) % 4
                    return edge_sb[(e, g)][:, a:a + 2, :]

                for it in range(ITERS):
                    # ---- step1: M_e[d, t] += h^T blocks @ edge_e
                    # (even e then odd e: one open accumulation group per
                    # PSUM bank; see module docstring)
                    mps = mps_pool.tile([D, E * S], dt.float32, tag="mps")
                    msgs_ps = sps_pool.tile([D, S], dt.float32, tag="sps")
                    for phase in range(2):
                        es = e_order[phase * 4:(phase + 1) * 4]
                        if fp8:
                            for p in range(NSB // 2):
                                for e in es:
                                    nc.tensor.matmul(
                                        mps[:, e * S:(e + 1) * S],
                                        hblk[p][:],
                                        edge_pair_ap(e, p),
                                        start=(p == 0), stop=(p == NSB // 2 - 1),
                                        perf_mode=PM.DoubleRow,
                                    )
                        else:
                            for sb in range(NSB):
                                for e in es:
                                    nc.tensor.matmul(
                                        mps[:, e * S:(e + 1) * S],
                                        hblk[sb][:],
                                        edge_ap(e, sb),
                                        start=(sb == 0), stop=(sb == NSB - 1),
                                    )
                        # step2 for this phase: msgs += W_e^T M_e
                        # (copies alternate DVE/ACT so neither engine gates)
                        for ei, e in enumerate(es):
                            m_bf = stpool.tile([D, S], dt.bfloat16,
                                               tag=f"mbf{e}", name=f"mbf{e}")
                            if ei % 2 == 0:
                                nc.vector.tensor_copy(
                                    m_bf[:], mps[:, e * S:(e + 1) * S])
                            else:
                                nc.scalar.activation(
                                    m_bf[:], mps[:, e * S:(e + 1) * S],
                                    AF.Copy)
                            nc.tensor.matmul(
                                msgs_ps[:], wmsg_sb[:, e * D:(e + 1) * D],
                                m_bf[:],
                                start=(e == 0), stop=(e == E - 1),
                            )
                    msgs_bf = spool.tile([D, S], dt.bfloat16, tag="msgsbf")
                    nc.vector.tensor_copy(msgs_bf[:], msgs_ps[:])

                    # ---- GRU on own shard, transposed [D, S] layout
                    new_hT = spool.tile([D, S], dt.bfloat16, tag="hT")
                    gate = []
                    for g in range(2):
                        gp = sps_pool.tile([D, S], dt.float32, tag="sps")
                        nc.tensor.matmul(gp[:], wi_sb[:, g * D:(g + 1) * D],
                                         msgs_bf[:], start=True, stop=False)
                        nc.tensor.matmul(gp[:], wh_sb[:, g * D:(g + 1) * D],
                                         hT[:], start=False, stop=True)
                        gs = stpool.tile([D, S], dt.float32, tag=f"g{g}")
                        nc.scalar.activation(gs[:], gp[:], AF.Sigmoid,
                                             bias=bias_sb[:, g:g + 1])
                        gate.append(gs)
                    r_g, z_g = gate

                    inp = sps_pool.tile([D, S], dt.float32, tag="sps")
                    nc.tensor.matmul(inp[:], wi_sb[:, 2 * D:3 * D], msgs_bf[:],
                                     start=True, stop=True)
                    hnp = sps_pool.tile([D, S], dt.float32, tag="sps")
                    nc.tensor.matmul(hnp[:], wh_sb[:, 2 * D:3 * D], hT[:],
                                     start=True, stop=True)
                    t1 = stpool.tile([D, S], dt.float32, tag="t1")
                    nc.vector.tensor_mul(t1[:], r_g[:], hnp[:])
                    t2 = stpool.tile([D, S], dt.float32, tag="t2")
                    nc.vector.tensor_add(t2[:], t1[:], inp[:])
                    n_sb = stpool.tile([D, S], dt.float32, tag="n")
                    nc.scalar.activation(n_sb[:], t2[:], AF.Tanh,
                                         bias=bias_sb[:, 2:3])
                    # h_new = n + z * (h - n)
                    d1 = stpool.tile([D, S], dt.float32, tag="d1")
                    nc.vector.tensor_sub(d1[:], hT[:], n_sb[:])
                    d2 = stpool.tile([D, S], dt.float32, tag="d2")
                    nc.vector.tensor_mul(d2[:], z_g[:], d1[:])
                    nc.vector.tensor_add(new_hT[:], n_sb[:], d2[:])
                    hT = new_hT

                    if it < ITERS - 1:
                        # transpose own h^T -> normal layout, AG, refill hblk
                        tp = sps_pool.tile([D, S], dt.bfloat16, tag="tp",
                                           bufs=1)
                        hsend = spool.tile([D, S], edt, tag="hsend")
                        for hh in range(2):
                            nc.tensor.transpose(
                                tp[:, hh * D:(hh + 1) * D],
                                hT[:, hh * D:(hh + 1) * D], id_sb[:])
                            nc.vector.tensor_copy(
                                hsend[:, hh * D:(hh + 1) * D],
                                tp[:, hh * D:(hh + 1) * D])
                        agin = dram.tile([S, D], edt, tag="agin")
                        for hh in range(2):
                            nc.sync.dma_start(
                                agin[hh * D:(hh + 1) * D, :],
                                hsend[:, hh * D:(hh + 1) * D])
                        agout = dram.tile([N, D], edt, tag="agout")
                        nc.gpsimd.collective_compute(
                            "AllGather", mybir.AluOpType.bypass,
                            replica_groups=RG,
                            ins=[agin.opt()], outs=[agout.opt()],
                        )
                        if fp8:
                            hblk = []
                            for p in range(NSB // 2):
                                t = hpool.tile([D, 2, D], edt,
                                               tag=f"hpair{p}",
                                               name=f"hpair{p}")
                                nc.sync.dma_start(
                                    t[:],
                                    agout[2 * p * D:(2 * p + 2) * D, :]
                                    .rearrange("(a p) c -> p a c", p=D))
                                hblk.append(t)
                        else:
                            hblk = []
                            for sb in range(NSB):
                                t = hpool.tile([D, D], edt, tag=f"hblk{sb}",
                                               name=f"hblk{sb}")
                                nc.sync.dma_start(
                                    t[:], agout[sb * D:(sb + 1) * D, :])
                                hblk.append(t)
                        # ~2.8us of throwaway matmuls during the AG so the
                        # PE's HAM clock gate stays at full rate (re-throttles
                        # after ~3.4us idle; the AG floor is ~4.6us, so these
                        # never delay step1)
                        junk = sps_pool.tile([D, 4 * D], dt.float32,
                                             tag="tp", bufs=1)
                        for _ in range(13):
                            nc.tensor.matmul(junk[:], id_sb[:],
                                             wmsg_sb[:, :4 * D],
                                             start=True, stop=True)

                # ---- final AG of h^T feeds the bilinear readout
                agin_t = dram.tile([D, S], dt.bfloat16, tag="agin_t")
                nc.sync.dma_start(agin_t[:], hT[:])
                agout_t = dram.tile([N_CORES * D, S], dt.bfloat16,
                                    tag="agout_t")
                nc.gpsimd.collective_compute(
                    "AllGather", mybir.AluOpType.bypass,
                    replica_groups=RG,
                    ins=[agin_t.opt()], outs=[agout_t.opt()],
                )
                hTf = spool.tile([D, N], dt.bfloat16, tag="hTf")
                for j in range(N_CORES):
                    nc.sync.dma_start(hTf[:, j * S:(j + 1) * S],
                                      agout_t[j * D:(j + 1) * D, :])

                # hA^T = A^T h^T (own shard) -- overlaps the AG
                hap = sps_pool.tile([D, S], dt.float32, tag="sps")
                nc.tensor.matmul(hap[:], aro_sb[:], hT[:], start=True,
                                 stop=True)
                hA_bf = spool.tile([D, S], dt.bfloat16, tag="hA")
                nc.vector.tensor_copy(hA_bf[:], hap[:])
                junk2 = sps_pool.tile([D, 4 * D], dt.float32, tag="tp",
                                      bufs=1)
                for _ in range(11):
                    nc.tensor.matmul(junk2[:], id_sb[:], wmsg_sb[:, :4 * D],
                                     start=True, stop=True)

                for isub in range(2):
                    for jc in range(N_CORES):
                        lp = sps_pool.tile([D, S], dt.float32, tag="sps")
                        nc.tensor.matmul(lp[:],
                                         hA_bf[:, isub * D:(isub + 1) * D],
                                         hTf[:, jc * S:(jc + 1) * S],
                                         start=True, stop=True)
                        ost = stpool.tile([D, S], dt.float32, tag="ost")
                        nc.vector.tensor_copy(ost[:], lp[:])
                        nc.sync.dma_start(
                            out.ap()[isub * D:(isub + 1) * D,
                                     jc * S:(jc + 1) * S],
                            ost[:])

    nc.compile()
    return nc


def make_in_maps(node_embeddings, edge_embeddings, W_msg, b_msg, Wi, Wh,
                 b_gru, A_readout, fp8=FP8):
    bf16 = ml_dtypes.bfloat16
    e4m3 = ml_dtypes.float8_e4m3
    wm = W_msg.transpose(1, 0, 2).reshape(D, E * D)
    if fp8:
        wm = wm / EDGE_SCALE  # undo the edge pre-scale
    wmsg = np.ascontiguousarray(wm).astype(bf16)
    wi_b = np.ascontiguousarray(Wi).astype(bf16)
    wh_b = np.ascontiguousarray(Wh).astype(bf16)
    # messages enter the GRU only through gi = (raw_msgs + b_msg) @ Wi + b_gru,
    # so fold b_msg into a per-gate bias (fp32, exact).
    b_eff = (b_msg.astype(np.float64) @ Wi.astype(np.float64)
             + b_gru.astype(np.float64)).astype(np.float32)
    bias = np.ascontiguousarray(b_eff.reshape(3, D).T)  # [D, 3]
    aro_b = np.ascontiguousarray(A_readout).astype(bf16)
    ident = np.eye(D, dtype=np.float32).astype(bf16)
    if fp8:
        h0n = np.ascontiguousarray(node_embeddings).astype(e4m3)
    else:
        h0n = np.ascontiguousarray(node_embeddings).astype(bf16)

    in_maps = []
    for k in range(N_CORES):
        sl = slice(k * S, (k + 1) * S)
        ek = np.ascontiguousarray(
            edge_embeddings[:, sl, :].transpose(2, 0, 1).reshape(E * N, S))
        if fp8:
            ek = (ek * EDGE_SCALE).astype(e4m3)
        else:
            ek = ek.astype(bf16)
        h0t = np.ascontiguousarray(node_embeddings[sl].T).astype(bf16)
        in_maps.append({
            "edgek": ek, "h0n": h0n, "h0t": h0t, "wmsg": wmsg, "wi": wi_b,
            "wh": wh_b, "bias": bias, "aro": aro_b, "ident": ident,
        })
    return in_maps


_cache = {}


def kernel(node_embeddings, edge_embeddings, W_msg, b_msg, Wi, Wh, b_gru,
           A_readout):
    if "nc" not in _cache:
        _cache["nc"] = build_nc(reps=1)
    nc = _cache["nc"]
    in_maps = make_in_maps(node_embeddings, edge_embeddings, W_msg, b_msg,
                           Wi, Wh, b_gru, A_readout)
    res = bass_utils.run_bass_kernel_spmd(
        nc, in_maps, core_ids=list(range(N_CORES)))
    return np.concatenate([res.results[k]["out"] for k in range(N_CORES)],
                          axis=0)
